# revision 1
# baseline (speedup 1.0000x reference)
"""Trainium2 Bass kernel for CrossDecoder kNN-mining margin loss.

Math: reference mines, per query q (both columns of train_ill), the k+1
nearest rows of X (rows = concat of both manifolds, dim 512) and uses the
*distances* from q to its own k nearest neighbours (self excluded) in a
margin loss.  Ranking and values only need, per query row, the top-(k+1)
smallest of  dist(q, j) = |q|^2 + |y_j|^2 - 2 q.y_j ; since |q|^2 is
row-constant we rank by  score(q,j) = 2 q.y_j - |y_j|^2  (descending) and
recover dist = |q|^2 - score on the host.

Device work (SPMD over 8 cores, candidate axis sharded 30000 -> 3750/core):
  - scores for a [128-query, 470-candidate] tile: the idle Scalar engine
    pre-writes -|y|^2 into the PSUM tile, then 4 accumulated K=128 fp16
    matmuls (queries pre-scaled by 2) add 2 q.y on top (start=False).
    The first 16 groups instead open with a start=True K=2 fp16 bias
    matmul: NEFF load clears PSUM has_written bits, and a start=False
    matmul on a cleared bit would overwrite the ACT-written bias.
  - nc.vector.max (top-8, descending) per chunk straight off PSUM.
Each core emits, per query, 8 chunks x top-8 = 64 candidate scores.
Host merges 8 cores x 64 = 512 candidates/row -> exact top-(k+1) w.p. 1
(would only fail if >8 of the true top-11 landed in one 470-wide chunk).
"""

import os
import numpy as np

M_, N_, D_, T_ = 2, 30000, 256, 3000
NCORES = 8
NSHARD = N_ // NCORES          # 3750
FCH = 470                      # candidate chunk width (>=256 keeps fp32r at full
                               # rate; must be EVEN: fp32r ISA requires even
                               # innermost free count on moving operand and dst)
NFC = 8                        # chunks per shard
NPAD = FCH * NFC               # 3760
KD = M_ * D_                   # 512 contraction dim
KCH = KD // 128                # 4 K-chunks
QT = 128                       # queries per tile (PSUM partition dim)
NQ = 6016                      # both query sets packed (6000) padded to 47 tiles
NQT = NQ // QT                 # 47 query tiles
QBLK = 4                       # query tiles per DMA block
NBLK = 12                      # 11 full blocks + one 3-tile block
PAD_SQY = 60000.0              # pad candidates rank last (fits fp16)

_cache = {}


def _build_program():
    import concourse.bass as bass
    import concourse.tile as tile
    from concourse import bacc, mybir

    dt = mybir.dt
    nc = bacc.Bacc(
        "TRN2", target_bir_lowering=False, debug=False, num_devices=NCORES
    )

    xq_d = nc.dram_tensor("xq", [KCH, 128, NQ], dt.float16, kind="ExternalInput")
    xs_d = nc.dram_tensor("xs", [KCH, 128, NPAD], dt.float16, kind="ExternalInput")
    # -|y|^2 bias, broadcast across partitions; the idle Scalar engine
    # copies it into each PSUM tile and the data matmuls (start=False)
    # accumulate on top, keeping the PE at its pure-matmul floor.
    sqyb_d = nc.dram_tensor("sqyb", [128, NPAD], dt.float32, kind="ExternalInput")
    # fp16 hi/lo bias rows + -1 weights for the first-16 "opener" groups
    sqy_d = nc.dram_tensor("sqy", [2, NPAD], dt.float16, kind="ExternalInput")
    neg1_d = nc.dram_tensor("neg1", [2, QT], dt.float16, kind="ExternalInput")
    cand_d = nc.dram_tensor("cand", [NBLK, 128, QBLK * 64], dt.float32,
                            kind="ExternalOutput")

    with tile.TileContext(nc) as tc:
        with (
            tc.tile_pool(name="resident", bufs=1) as res_pool,
            tc.tile_pool(name="xq", bufs=2) as xq_pool,
            tc.tile_pool(name="cand", bufs=2) as cand_pool,
            tc.tile_pool(name="psum", bufs=8, space=bass.MemorySpace.PSUM) as psum_pool,
        ):
            xs_sb = [res_pool.tile([128, NPAD], dt.float16, tag=f"xs{k}", name=f"xs_sb{k}")
                     for k in range(KCH)]
            for k in range(KCH):
                nc.sync.dma_start(out=xs_sb[k][:, :], in_=xs_d[k])
            sqyb_sb = res_pool.tile([128, NPAD], dt.float32, tag="sqyb")
            nc.sync.dma_start(out=sqyb_sb[:, :], in_=sqyb_d[:, :])
            sqy_sb = res_pool.tile([2, NPAD], dt.float16, tag="sqy")
            nc.sync.dma_start(out=sqy_sb[:, :], in_=sqy_d[:, :])
            neg1_sb = res_pool.tile([2, QT], dt.float16, tag="neg1")
            nc.sync.dma_start(out=neg1_sb[:, :], in_=neg1_d[:, :])

            # Pre-bias each PSUM tile with -|y|^2 on the (otherwise idle)
            # Scalar engine, then accumulate the four data matmuls on top
            # with start=False (skip_group_check: the group is opened by the
            # ACT write, which the group checker doesn't know about).
            from concourse import mybir as _mb
            ui = 0
            for blk in range(NBLK):
                q0 = blk * QBLK * QT
                nqt = min(QBLK, NQT - blk * QBLK)
                xq_sb = [xq_pool.tile([128, nqt * QT], dt.float16,
                                      tag=f"xq{k}", name=f"xq_sb{k}")
                         for k in range(KCH)]
                for k in range(KCH):
                    nc.sync.dma_start(out=xq_sb[k][:, :],
                                      in_=xq_d[k, :, q0:q0 + nqt * QT])
                cand_sb = cand_pool.tile([128, nqt * 64], dt.float32, tag="cand")
                for j in range(nqt):
                    for f in range(NFC):
                        ps = psum_pool.tile([128, FCH], dt.float32, tag="ps")
                        # NEFF load clears PSUM has_written bits; a start=False
                        # matmul on a cleared bit OVERWRITES instead of
                        # accumulating, which would discard the ACT-written
                        # bias. The first 16 groups (>= 2 full trips through
                        # the 8 PSUM slots) open with a start=True matmul to
                        # set the bits; afterwards they stay set for the rest
                        # of the kernel and the cheap ACT pre-bias is safe.
                        if ui < 16:
                            nc.tensor.matmul(
                                ps[:, :], lhsT=neg1_sb[:, :],
                                rhs=sqy_sb[:, f * FCH:(f + 1) * FCH],
                                start=True, stop=False,
                            )
                        else:
                            nc.scalar.activation(
                                ps[:, :], sqyb_sb[:, f * FCH:(f + 1) * FCH],
                                _mb.ActivationFunctionType.Copy,
                            )
                        ui += 1
                        for k in range(KCH):
                            nc.tensor.matmul(
                                ps[:, :],
                                lhsT=xq_sb[k][:, j * QT:(j + 1) * QT],
                                rhs=xs_sb[k][:, f * FCH:(f + 1) * FCH],
                                start=False,
                                stop=(k == KCH - 1),
                                skip_group_check=True,
                            )
                        o = j * 64 + f * 8
                        nc.vector.max(cand_sb[:, o:o + 8], ps[:, :])
                nc.sync.dma_start(out=cand_d[blk, :, :nqt * 64],
                                  in_=cand_sb[:, :])

    nc.compile()
    return nc


def _get_program():
    if "nc" not in _cache:
        _cache["nc"] = _build_program()
    return _cache["nc"]


def _prep_inputs(X, left, right):
    """X: [N, 512] fp32. Returns (shared xq map entries, per-core xs/sqy)."""
    q_idx = np.concatenate([right, left, np.zeros(NQ - 2 * T_, np.int64)])
    Xq = (2.0 * X[q_idx]).astype(np.float16)
    Xq[2 * T_:] = 0.0
    xq_in = np.ascontiguousarray(Xq.T.reshape(KCH, 128, NQ))

    per_core = []
    for corei in range(NCORES):
        shard = X[corei * NSHARD:(corei + 1) * NSHARD]          # [3750, 512]
        xs = np.zeros((KD, NPAD), np.float16)
        xs[:, :NSHARD] = shard.T.astype(np.float16)
        sqy = np.full(NPAD, PAD_SQY, np.float32)
        sqy[:NSHARD] = (shard.astype(np.float64) ** 2).sum(1).astype(np.float32)
        sqy_hi = sqy.astype(np.float16)
        sqy_lo = (sqy - sqy_hi.astype(np.float32)).astype(np.float16)
        per_core.append({
            "xq": xq_in,
            "xs": np.ascontiguousarray(xs.reshape(KCH, 128, NPAD)),
            "sqyb": np.ascontiguousarray(np.broadcast_to(-sqy, (128, NPAD))).astype(np.float32),
            "sqy": np.stack([sqy_hi, sqy_lo]),
            "neg1": np.full((2, QT), -1.0, np.float16),
        })
    return per_core


def _mine_scores(in_maps, trace=False):
    from concourse.bass_utils import run_bass_kernel_spmd

    nc = _get_program()
    try:
        res = run_bass_kernel_spmd(nc, in_maps, list(range(NCORES)), trace=trace)
    except Exception:
        if not trace:
            raise
        res = run_bass_kernel_spmd(nc, in_maps, list(range(NCORES)), trace=False)
    _cache["last_result"] = res
    # per-core cand: [NBLK, 128, QBLK*64] -> [NQ, 64]
    cores = []
    for i in range(NCORES):
        c = res.results[i]["cand"].reshape(NBLK, 128, QBLK, 64)
        cores.append(c.transpose(0, 2, 1, 3).reshape(NBLK * QBLK * 128, 64)[:NQ])
    return np.concatenate(cores, axis=1)                         # [NQ, 512]


def kernel(outlayer, c, train_ill, k):
    k = int(k)
    outlayer = np.asarray(outlayer, np.float32)
    train_ill = np.asarray(train_ill)
    X = np.ascontiguousarray(
        outlayer.transpose(1, 0, 2).reshape(N_, KD)).astype(np.float32)
    left = train_ill[:, 0].astype(np.int64)
    right = train_ill[:, 1].astype(np.int64)

    in_maps = _prep_inputs(X, left, right)
    scores = _mine_scores(in_maps, trace=bool(int(os.environ.get("KNN_TRACE", "0"))))

    # top-(k+1) scores (descending) per query row; row 0 is the self match.
    nkeep = k + 1
    part = np.partition(scores, scores.shape[1] - nkeep, axis=1)[:, -nkeep:]
    top = np.sort(part, axis=1)[:, ::-1]                         # [NQ, k+1]

    X64 = X.astype(np.float64)
    sq = (X64 ** 2).sum(1)                                       # [N]

    s_right = top[:T_]                                           # mining of right idx
    s_left = top[T_:2 * T_]                                      # mining of left idx

    # B[i, j] = dist(q_i, j-th NN of q_i) = |q_i|^2 - score, self (col 0) dropped
    B2 = sq[right][:, None] - s_right[:, 1:].astype(np.float64)
    B1 = sq[left][:, None] - s_left[:, 1:].astype(np.float64)

    D = ((X64[left] - X64[right]) ** 2).sum(1) + 1.0             # [t]
    L1 = np.maximum(D[:, None] - B1, 0.0)
    L2 = np.maximum(D[:, None] - B2, 0.0)
    loss = (L1.mean() + L2.mean()) / 2.0
    return np.asarray(loss, dtype=np.float32)



# revision 11
# speedup vs baseline: 1.4038x; 1.4038x over previous
"""Trainium2 Bass kernel for CrossDecoder kNN-mining margin loss (fp8 edition).

Math: the reference mines, per query q (both columns of train_ill), the k+1
nearest of N=30000 candidates under sum-of-manifolds squared distance
(concat dim 512), then uses those neighbour distances in a margin loss.

Device strategy (SPMD over 8 cores):
  - Rank candidates by score(q,j) = 2q.y_j - |y_j|^2 (descending) computed in
    fp8 (e4m3) DoubleRow matmuls (2 accumulated K=256 matmuls per 256-column
    chunk) -> PSUM fp32.  fp8 adds ~2 abs noise, so the device returns window
    INDICES and the host exact-recomputes the selected windows, making the
    final top-k exact w.h.p. (validated ~1e-16 in simulation).
  - Candidates are globally sorted by |y|^2 and grouped into windows of 16
    with near-constant norm; windows are dealt round-robin to the 8 cores.
    The matmul computes UNBIASED 2q.y; the per-window bias -|y_w|^2 is
    subtracted after windowed max-pooling, so the tensor engine runs at its
    pure-matmul floor (no per-tile bias rewrite).
  - PSUM egress is the bottleneck (only ACT and DVE can read PSUM): per
    128-query tile the 8 PSUM banks (4096 candidates) are split as 4
    two-bank tiles: 3 are copied out by ACT (bitcast to uint64 to halve the
    element count) and max-pooled by the Pool engine (pairwise-max trees);
    the 4th is windowed-max-reduced by DVE straight from PSUM.  Stage 2 on
    the 256 window maxima: Pool subtracts the window bias, DVE extracts
    top-8 values + window indices (max8 / max_index).
Host merges 8 cores x 8 windows x 47 tiles, picks top SELW windows per query
by device value, exact-recomputes those SELW*16 candidate distances in fp64,
and forms the margin loss exactly as the reference does.
"""

import os
import numpy as np

M_, N_, D_, T_ = 2, 30000, 256, 3000
NCORES = 8
KD = M_ * D_                   # 512 contraction dim
QT = 128                       # queries per tile (PSUM partition dim)
NQ = 6016                      # 2*T_ queries padded to 47 tiles
NQT = NQ // QT                 # 47
QBLK = 4                       # query tiles per output block
NBLK = 12                      # 11 full blocks + one 3-tile block
TBW = 512                      # columns per PSUM bank tile
NTB = 8                        # bank tiles per query tile
NCP = 6                        # banks ACT-copies out; the rest DVE-reduces
CC = 256                       # matmul column chunk (DR moving limit 512)
NPAD = TBW * NTB               # 4096 candidate slots per core
W = 16                         # window width (norm-sorted candidates)
NWIN = NPAD // W               # 256 windows per core
NSUP = NWIN // 2               # 128 superwindows (top-8 mined at W=32)
GWIN = N_ // W                 # 1875 global windows (30000 = 1875*16)
SELW = 14                      # superwindows/query the host exact-recomputes
PAD_BIAS = 30000.0             # pad windows rank last

_cache = {}


def _build_program():
    import concourse.bass as bass
    import concourse.tile as tile
    from concourse import bacc, mybir

    dt = mybir.dt
    nc = bacc.Bacc(
        "TRN2", target_bir_lowering=False, debug=False, num_devices=NCORES
    )

    xq_d = nc.dram_tensor("xq", [128, 4, NQ], dt.float8e4, kind="ExternalInput")
    xs_d = nc.dram_tensor("xs", [NTB, 128, 4, TBW], dt.float8e4,
                          kind="ExternalInput")
    bias_d = nc.dram_tensor("bias", [128, NWIN], dt.bfloat16,
                            kind="ExternalInput")
    # per query tile: 8 bf16 top values (bitcast as u16) + 8 u16 window ids
    out_d = nc.dram_tensor("out", [NBLK, 128, QBLK * 16], dt.uint16,
                           kind="ExternalOutput")

    DR = mybir.MatmulPerfMode.DoubleRow
    MAX = mybir.AluOpType.max

    with tile.TileContext(nc) as tc:
        with (
            tc.tile_pool(name="res", bufs=1) as res_pool,
            tc.tile_pool(name="cp", bufs=3) as cp_pool,
            tc.tile_pool(name="work", bufs=3) as work_pool,
            tc.tile_pool(name="out", bufs=2) as out_pool,
            tc.tile_pool(name="psum", bufs=4, space=bass.MemorySpace.PSUM) as psum_pool,
        ):
            xs_sb = [res_pool.tile([128, 4, TBW], dt.float8e4, tag=f"xs{t}",
                                   name=f"xs_sb{t}")
                     for t in range(NTB)]
            for t in range(NTB):
                nc.sync.dma_start(out=xs_sb[t][:, :, :], in_=xs_d[t])
            bias_sb = res_pool.tile([128, NWIN], dt.bfloat16, tag="bias")
            nc.sync.dma_start(out=bias_sb[:, :], in_=bias_d[:, :])
            # resident queries, DMA'd in blocks so the first tile starts early
            xq_sb = res_pool.tile([128, 4, NQ], dt.float8e4, tag="xq")
            for blk in range(NBLK):
                q0 = blk * QBLK * QT
                q1 = min(NQ, q0 + QBLK * QT)
                nc.sync.dma_start(out=xq_sb[:, :, q0:q1],
                                  in_=xq_d[:, :, q0:q1])

            for blk in range(NBLK):
                nqt = min(QBLK, NQT - blk * QBLK)
                out_sb = out_pool.tile([128, nqt * 16], dt.uint16, tag="out")
                for jj in range(nqt):
                    j = blk * QBLK + jj
                    # Pool/gpsimd has no compute and cannot read PSUM on
                    # TRN2; scan split: units 0-2 ACT-copied to bf16 SBUF
                    # (one fused DVE max tree over all three), unit 3
                    # windowed-max'd by DVE straight from PSUM.
                    win = work_pool.tile([128, NWIN], dt.bfloat16, tag="win")
                    cp = cp_pool.tile([128, NCP * TBW], dt.bfloat16, tag="cp")
                    for tb in range(NTB):
                        ps = psum_pool.tile([128, TBW], dt.float32, tag="ps")
                        for cc in range(TBW // CC):
                            for p in range(2):
                                nc.tensor.matmul(
                                    ps[:, cc * CC:(cc + 1) * CC],
                                    lhsT=xq_sb[:, 2 * p:2 * p + 2,
                                               j * QT:(j + 1) * QT],
                                    rhs=xs_sb[tb][:, 2 * p:2 * p + 2,
                                                  cc * CC:(cc + 1) * CC],
                                    start=(p == 0), stop=(p == 1),
                                    perf_mode=DR,
                                    # 2nd 256-col group shares the PSUM bank;
                                    # the sim's group checker is bank-granular
                                    skip_group_check=(cc == 1),
                                )
                        if tb < NCP:
                            nc.scalar.activation(
                                cp[:, tb * TBW:(tb + 1) * TBW], ps[:, :],
                                mybir.ActivationFunctionType.Copy)
                        else:
                            ps3 = ps[:, :].rearrange("p (w j) -> p w j", j=W)
                            w0 = tb * (TBW // W)
                            nc.vector.tensor_reduce(
                                out=win[:, w0:w0 + TBW // W],
                                in_=ps3, axis=mybir.AxisListType.X, op=MAX)
                    # fused 4-level pairwise-max tree over the copied banks
                    NW3 = NCP * TBW // W                     # 192 windows
                    c3 = cp[:, :].rearrange("p (w j) -> p w j", j=W)
                    t1 = work_pool.tile([128, NW3, 8], dt.bfloat16, tag="t1")
                    nc.vector.tensor_tensor(
                        out=t1[:, :, :], in0=c3[:, :, 0:8],
                        in1=c3[:, :, 8:16], op=MAX)
                    t2 = work_pool.tile([128, NW3, 4], dt.bfloat16, tag="t2")
                    nc.vector.tensor_tensor(
                        out=t2[:, :, :], in0=t1[:, :, 0:4],
                        in1=t1[:, :, 4:8], op=MAX)
                    t3 = work_pool.tile([128, NW3, 2], dt.bfloat16, tag="t3")
                    nc.vector.tensor_tensor(
                        out=t3[:, :, :], in0=t2[:, :, 0:2],
                        in1=t2[:, :, 2:4], op=MAX)
                    nc.vector.tensor_tensor(
                        out=win[:, 0:NW3], in0=t3[:, :, 0], in1=t3[:, :, 1],
                        op=MAX)
                    biased = work_pool.tile([128, NWIN], dt.bfloat16,
                                            tag="biased")
                    nc.vector.tensor_sub(biased[:, :], win[:, :],
                                         bias_sb[:, :])
                    # extra pairwise level: superwindows of 2 windows (W=32)
                    sup = work_pool.tile([128, NSUP], dt.bfloat16, tag="sup")
                    b3 = biased[:, :].rearrange("p (s two) -> p s two", two=2)
                    nc.vector.tensor_tensor(
                        out=sup[:, :], in0=b3[:, :, 0], in1=b3[:, :, 1],
                        op=MAX)
                    vals8 = out_sb[:, jj * 16:jj * 16 + 8].bitcast(dt.bfloat16)
                    idx8 = out_sb[:, jj * 16 + 8:jj * 16 + 16]
                    nc.vector.max(vals8, sup[:, :])
                    nc.vector.max_index(idx8, vals8, sup[:, :])
                nc.sync.dma_start(out=out_d[blk, :, :nqt * 16],
                                  in_=out_sb[:, :])

    nc.compile()
    return nc


def _get_program():
    if "nc" not in _cache:
        _cache["nc"] = _build_program()
    return _cache["nc"]


def _prep_inputs(X, left, right):
    """X: [N, 512] fp32.  Returns (per-core input maps, order_norm)."""
    import ml_dtypes
    f8 = ml_dtypes.float8_e4m3

    sq = (X.astype(np.float64) ** 2).sum(1)
    order_norm = np.argsort(sq, kind="stable")          # ascending norm

    q_idx = np.concatenate([right, left, np.zeros(NQ - 2 * T_, np.int64)])
    Xq8 = (2.0 * X[q_idx]).astype(f8)
    Xq8[2 * T_:] = 0.0
    # [NQ, 512] -> [128, 4, NQ]: feature kk*128+r at [r, kk, q]
    xq_in = np.ascontiguousarray(
        Xq8.T.reshape(4, 128, NQ).transpose(1, 0, 2))

    Xs8 = X[order_norm].astype(f8)                      # sorted candidates
    sqw = sq[order_norm].reshape(GWIN, W).mean(1)       # per-window bias

    per_core = []
    for c in range(NCORES):
        # core c holds global windows g = 8w + c, w = 0..NWIN-1 (g < GWIN)
        gws = 8 * np.arange(NWIN) + c
        valid = gws < GWIN
        xs = np.zeros((NPAD, KD), f8)
        cand_rows = (gws[valid][:, None] * W + np.arange(W)[None, :]).ravel()
        xs[: valid.sum() * W] = Xs8[cand_rows]
        xs_in = np.ascontiguousarray(
            xs.T.reshape(4, 128, NTB, TBW).transpose(2, 1, 0, 3))
        bias = np.full(NWIN, PAD_BIAS, np.float32)
        bias[valid] = sqw[gws[valid]].astype(np.float32)
        bias_in = np.broadcast_to(
            bias.astype(ml_dtypes.bfloat16), (128, NWIN)).copy()
        per_core.append({"xq": xq_in, "xs": xs_in, "bias": bias_in})
    return per_core, order_norm


def _mine(in_maps, trace=False):
    from concourse.bass_utils import run_bass_kernel_spmd
    import ml_dtypes

    nc = _get_program()
    try:
        res = run_bass_kernel_spmd(nc, in_maps, list(range(NCORES)), trace=trace)
    except Exception:
        if not trace:
            raise
        res = run_bass_kernel_spmd(nc, in_maps, list(range(NCORES)), trace=False)
    _cache["last_result"] = res
    vals = np.empty((NQ, NCORES * 8), np.float32)
    sups = np.empty((NQ, NCORES * 8), np.int64)
    cores = np.empty((NQ, NCORES * 8), np.int64)
    for c in range(NCORES):
        o = res.results[c]["out"].reshape(NBLK, 128, QBLK, 16)
        o = o.transpose(0, 2, 1, 3).reshape(NBLK * QBLK * 128, 16)[:NQ]
        vals[:, c * 8:(c + 1) * 8] = (
            o[:, :8].view(ml_dtypes.bfloat16).astype(np.float32))
        sups[:, c * 8:(c + 1) * 8] = o[:, 8:16].astype(np.int64)
        cores[:, c * 8:(c + 1) * 8] = c
    return vals, sups, cores


def kernel(outlayer, c, train_ill, k):
    k = int(k)
    outlayer = np.asarray(outlayer, np.float32)
    train_ill = np.asarray(train_ill)
    X = np.ascontiguousarray(
        outlayer.transpose(1, 0, 2).reshape(N_, KD)).astype(np.float32)
    left = train_ill[:, 0].astype(np.int64)
    right = train_ill[:, 1].astype(np.int64)

    in_maps, order_norm = _prep_inputs(X, left, right)
    vals, sups, cores = _mine(
        in_maps, trace=bool(int(os.environ.get("KNN_TRACE", "0"))))
    vals = vals[:2 * T_]
    sups = sups[:2 * T_]
    cores = cores[:2 * T_]

    # host: pick SELW superwindows per query by device value; each expands to
    # 2 windows of W sorted candidates; exact-recompute those
    nq = 2 * T_
    sel = np.argpartition(-vals, SELW - 1, axis=1)[:, :SELW]     # [nq, SELW]
    ssel = np.take_along_axis(sups, sel, axis=1)                 # supwin slots
    csel = np.take_along_axis(cores, sel, axis=1)                # owning core
    # windows w = 2s, 2s+1; global window g = 8w + core
    gsel = (8 * (2 * ssel[:, :, None] + np.arange(2)[None, None, :])
            + csel[:, :, None]).reshape(nq, SELW * 2)            # [nq, 2*SELW]
    cand_sorted = gsel[:, :, None] * W + np.arange(W)[None, None, :]
    cand_sorted = cand_sorted.reshape(nq, SELW * 2 * W)
    pad_mask = gsel.repeat(W).reshape(nq, SELW * 2 * W) >= GWIN
    cand = order_norm[np.clip(cand_sorted, 0, N_ - 1)]           # original ids
    q_idx = np.concatenate([right, left])

    X64 = X.astype(np.float64)
    sq = (X64 ** 2).sum(1)
    # exact sqdist via |q|^2 + |y|^2 - 2 q.y with per-chunk batched GEMV
    ncand = SELW * 2 * W
    B = np.empty((nq, ncand))
    step = 512
    for s in range(0, nq, step):
        e = min(s + step, nq)
        Y = X[cand[s:e]].astype(np.float64)                      # [b, nc, 512]
        G = np.einsum("bd,bcd->bc", X64[q_idx[s:e]], Y, optimize=True)
        B[s:e] = sq[q_idx[s:e], None] + sq[cand[s:e]] - 2.0 * G
    B[pad_mask] = np.inf
    B[cand == q_idx[:, None]] = np.inf                           # drop self
    B = np.sort(B, axis=1)[:, :k]                                # k NN dists

    D = ((X64[left] - X64[right]) ** 2).sum(1) + 1.0             # [t]
    B2 = B[:T_]                                                  # mining of right
    B1 = B[T_:]                                                  # mining of left
    L1 = np.maximum(D[:, None] - B1, 0.0)
    L2 = np.maximum(D[:, None] - B2, 0.0)
    loss = (L1.mean() + L2.mean()) / 2.0
    return np.asarray(loss, dtype=np.float32)


# revision 14
# speedup vs baseline: 1.7000x; 1.2110x over previous
"""Trainium2 Bass kernel for CrossDecoder kNN-mining margin loss (fp8 edition).

Math: the reference mines, per query q (both columns of train_ill), the k+1
nearest of N=30000 candidates under sum-of-manifolds squared distance
(concat dim 512), then uses those neighbour distances in a margin loss.

Device strategy (SPMD over 8 cores):
  - Rank candidates by score(q,j) = 2q.y_j - |y_j|^2 (descending) computed in
    fp8 (e4m3) DoubleRow matmuls (2 accumulated K=256 matmuls per 256-column
    chunk) -> PSUM fp32.  fp8 adds ~2 abs noise, so the device returns window
    INDICES and the host exact-recomputes the selected windows, making the
    final top-k near-exact (measured rel err ~5e-5).
  - Candidates are globally sorted by |y|^2 and grouped into windows of 32
    with near-constant norm; windows are dealt round-robin to the 8 cores.
    The matmul computes UNBIASED 2q.y; the per-window bias -|y_w|^2 is
    subtracted after windowed max-pooling, so the tensor engine runs at its
    pure-matmul floor (no per-tile bias rewrite).
  - PSUM egress is the bottleneck (only ACT and DVE can read PSUM on TRN2;
    Pool has no compute and cannot touch PSUM; no PSUM access pattern may
    cross a bank boundary or the device hard-faults).  Per 128-query tile
    the 3840 candidates sit in 7 full PSUM banks + 1 half bank: banks 0-1
    are windowed-max-reduced by DVE straight from PSUM, banks 2-7 are
    ACT-copied to bf16 SBUF and max-pooled by a fused 5-level DVE
    pairwise-max tree (tensor_tensor runs 2x on bf16).  Stage 2: DVE
    subtracts the window bias and extracts top-8 values + window indices
    (max8 / max_index) from the 120 window maxima.
Host merges 8 cores x 8 windows x 47 tiles, picks top SELW windows per query
by device value, exact-recomputes those SELW*32 candidate distances in fp64,
and forms the margin loss exactly as the reference does.
"""

import os
import numpy as np

M_, N_, D_, T_ = 2, 30000, 256, 3000
NCORES = 8
KD = M_ * D_                   # 512 contraction dim
QT = 128                       # queries per tile (PSUM partition dim)
NQ = 6016                      # 2*T_ queries padded to 47 tiles
NQT = NQ // QT                 # 47
QBLK = 4                       # query tiles per output block
NBLK = 12                      # 11 full blocks + one 3-tile block
CC = 256                       # matmul column chunk (DR moving limit 512)
# per-tile candidate banks: widths and scan policy (d=DVE direct, c=copied)
TBS = (512, 512, 512, 512, 512, 512, 512, 256)
TBPOL = ("d", "d", "c", "c", "c", "c", "c", "c")
NPAD = sum(TBS)                # 3840 candidate slots per core
NDIR = 1024                    # leading direct-scanned columns
NCPY = NPAD - NDIR             # 2816 ACT-copied columns
W = 32                         # window width (norm-sorted candidates)
NWIN = NPAD // W               # 120 windows per core
GWIN = (N_ + W - 1) // W       # 938 global windows (last one half real)
SELW = 14                      # windows per query the host exact-recomputes
PAD_BIAS = 30000.0             # pad windows rank last

_cache = {}


def _build_program():
    import concourse.bass as bass
    import concourse.tile as tile
    from concourse import bacc, mybir

    dt = mybir.dt
    nc = bacc.Bacc(
        "TRN2", target_bir_lowering=False, debug=False, num_devices=NCORES
    )

    xq_d = nc.dram_tensor("xq", [128, 4, NQ], dt.float8e4, kind="ExternalInput")
    xs_d = nc.dram_tensor("xs", [128, 4, NPAD], dt.float8e4,
                          kind="ExternalInput")
    bias_d = nc.dram_tensor("bias", [128, NWIN], dt.bfloat16,
                            kind="ExternalInput")
    # per query tile: 8 bf16 top values (bitcast as u16) + 8 u16 window ids
    out_d = nc.dram_tensor("out", [NBLK, 128, QBLK * 16], dt.uint16,
                           kind="ExternalOutput")

    DR = mybir.MatmulPerfMode.DoubleRow
    MAX = mybir.AluOpType.max
    TBO = [sum(TBS[:i]) for i in range(len(TBS))]        # column offsets

    with tile.TileContext(nc) as tc:
        with (
            tc.tile_pool(name="res", bufs=1) as res_pool,
            tc.tile_pool(name="cp", bufs=3) as cp_pool,
            tc.tile_pool(name="work", bufs=3) as work_pool,
            tc.tile_pool(name="out", bufs=2) as out_pool,
            tc.tile_pool(name="psum", bufs=8, space=bass.MemorySpace.PSUM) as psum_pool,
        ):
            xs_sb = res_pool.tile([128, 4, NPAD], dt.float8e4, tag="xs")
            for t in range(len(TBS)):
                nc.sync.dma_start(out=xs_sb[:, :, TBO[t]:TBO[t] + TBS[t]],
                                  in_=xs_d[:, :, TBO[t]:TBO[t] + TBS[t]])
            bias_sb = res_pool.tile([128, NWIN], dt.bfloat16, tag="bias")
            nc.sync.dma_start(out=bias_sb[:, :], in_=bias_d[:, :])
            # resident queries, DMA'd in blocks so the first tile starts early
            xq_sb = res_pool.tile([128, 4, NQ], dt.float8e4, tag="xq")
            for blk in range(NBLK):
                q0 = blk * QBLK * QT
                q1 = min(NQ, q0 + QBLK * QT)
                nc.sync.dma_start(out=xq_sb[:, :, q0:q1],
                                  in_=xq_d[:, :, q0:q1])

            for blk in range(NBLK):
                nqt = min(QBLK, NQT - blk * QBLK)
                out_sb = out_pool.tile([128, nqt * 16], dt.uint16, tag="out")
                for jj in range(nqt):
                    j = blk * QBLK + jj
                    win = work_pool.tile([128, NWIN], dt.bfloat16, tag="win")
                    cp = cp_pool.tile([128, NCPY], dt.bfloat16, tag="cp")
                    for tb, (tw, pol) in enumerate(zip(TBS, TBPOL)):
                        # uniform 512-col PSUM tiles (one per bank); the last
                        # 256-col unit just uses half its bank
                        ps = psum_pool.tile([128, 512], dt.float32, tag="ps")
                        for cc in range(tw // CC):
                            for p in range(2):
                                nc.tensor.matmul(
                                    ps[:, cc * CC:(cc + 1) * CC],
                                    lhsT=xq_sb[:, 2 * p:2 * p + 2,
                                               j * QT:(j + 1) * QT],
                                    rhs=xs_sb[:, 2 * p:2 * p + 2,
                                              TBO[tb] + cc * CC:
                                              TBO[tb] + (cc + 1) * CC],
                                    start=(p == 0), stop=(p == 1),
                                    perf_mode=DR,
                                    # 2nd 256-col group shares the PSUM bank;
                                    # the sim's group checker is bank-granular
                                    skip_group_check=(cc == 1),
                                )
                        if pol == "d":
                            ps3 = ps[:, 0:tw].rearrange("p (w j) -> p w j",
                                                        j=W)
                            w0 = TBO[tb] // W
                            nc.vector.tensor_reduce(
                                out=win[:, w0:w0 + tw // W],
                                in_=ps3, axis=mybir.AxisListType.X, op=MAX)
                        else:
                            o = TBO[tb] - NDIR
                            nc.scalar.activation(
                                cp[:, o:o + tw], ps[:, 0:tw],
                                mybir.ActivationFunctionType.Copy)
                    # fused 5-level pairwise-max tree over the copied banks
                    NWC = NCPY // W                          # 88 windows
                    c3 = cp[:, :].rearrange("p (w j) -> p w j", j=W)
                    t1 = work_pool.tile([128, NWC, 16], dt.bfloat16, tag="t1")
                    nc.vector.tensor_tensor(
                        out=t1[:, :, :], in0=c3[:, :, 0:16],
                        in1=c3[:, :, 16:32], op=MAX)
                    t2 = work_pool.tile([128, NWC, 8], dt.bfloat16, tag="t2")
                    nc.vector.tensor_tensor(
                        out=t2[:, :, :], in0=t1[:, :, 0:8],
                        in1=t1[:, :, 8:16], op=MAX)
                    t3 = work_pool.tile([128, NWC, 4], dt.bfloat16, tag="t3")
                    nc.vector.tensor_tensor(
                        out=t3[:, :, :], in0=t2[:, :, 0:4],
                        in1=t2[:, :, 4:8], op=MAX)
                    t4 = work_pool.tile([128, NWC, 2], dt.bfloat16, tag="t4")
                    nc.vector.tensor_tensor(
                        out=t4[:, :, :], in0=t3[:, :, 0:2],
                        in1=t3[:, :, 2:4], op=MAX)
                    nc.vector.tensor_tensor(
                        out=win[:, NDIR // W:], in0=t4[:, :, 0],
                        in1=t4[:, :, 1], op=MAX)
                    biased = work_pool.tile([128, NWIN], dt.bfloat16,
                                            tag="biased")
                    nc.vector.tensor_sub(biased[:, :], win[:, :],
                                         bias_sb[:, :])
                    vals8 = out_sb[:, jj * 16:jj * 16 + 8].bitcast(dt.bfloat16)
                    idx8 = out_sb[:, jj * 16 + 8:jj * 16 + 16]
                    nc.vector.max(vals8, biased[:, :])
                    nc.vector.max_index(idx8, vals8, biased[:, :])
                nc.sync.dma_start(out=out_d[blk, :, :nqt * 16],
                                  in_=out_sb[:, :])

    nc.compile()
    return nc


def _get_program():
    if "nc" not in _cache:
        _cache["nc"] = _build_program()
    return _cache["nc"]


def _prep_inputs(X, left, right):
    """X: [N, 512] fp32.  Returns (per-core input maps, order_norm)."""
    import ml_dtypes
    f8 = ml_dtypes.float8_e4m3

    sq = (X.astype(np.float64) ** 2).sum(1)
    order_norm = np.argsort(sq, kind="stable")          # ascending norm

    q_idx = np.concatenate([right, left, np.zeros(NQ - 2 * T_, np.int64)])
    Xq8 = (2.0 * X[q_idx]).astype(f8)
    Xq8[2 * T_:] = 0.0
    # [NQ, 512] -> [128, 4, NQ]: feature kk*128+r at [r, kk, q]
    xq_in = np.ascontiguousarray(
        Xq8.T.reshape(4, 128, NQ).transpose(1, 0, 2))

    Xs8 = np.zeros((GWIN * W, KD), f8)                  # sorted + tail pad
    Xs8[:N_] = X[order_norm].astype(f8)
    sq_sorted = np.full(GWIN * W, np.nan)
    sq_sorted[:N_] = sq[order_norm]
    # per-window bias: mean over REAL members
    sqw = np.nanmean(sq_sorted.reshape(GWIN, W), axis=1)

    per_core = []
    for c in range(NCORES):
        # core c holds global windows g = 8w + c, w = 0..NWIN-1 (g < GWIN)
        gws = 8 * np.arange(NWIN) + c
        valid = gws < GWIN
        xs = np.zeros((NPAD, KD), f8)
        cand_rows = (gws[valid][:, None] * W + np.arange(W)[None, :]).ravel()
        xs[: valid.sum() * W] = Xs8[cand_rows]
        xs_in = np.ascontiguousarray(
            xs.T.reshape(4, 128, NPAD).transpose(1, 0, 2))
        bias = np.full(NWIN, PAD_BIAS, np.float32)
        bias[valid] = sqw[gws[valid]].astype(np.float32)
        bias_in = np.broadcast_to(
            bias.astype(ml_dtypes.bfloat16), (128, NWIN)).copy()
        per_core.append({"xq": xq_in, "xs": xs_in, "bias": bias_in})
    return per_core, order_norm


def _mine(in_maps, trace=False):
    from concourse.bass_utils import run_bass_kernel_spmd
    import ml_dtypes

    nc = _get_program()
    try:
        res = run_bass_kernel_spmd(nc, in_maps, list(range(NCORES)), trace=trace)
    except Exception:
        if not trace:
            raise
        res = run_bass_kernel_spmd(nc, in_maps, list(range(NCORES)), trace=False)
    _cache["last_result"] = res
    vals = np.empty((NQ, NCORES * 8), np.float32)
    wins = np.empty((NQ, NCORES * 8), np.int64)
    for c in range(NCORES):
        o = res.results[c]["out"].reshape(NBLK, 128, QBLK, 16)
        o = o.transpose(0, 2, 1, 3).reshape(NBLK * QBLK * 128, 16)[:NQ]
        vals[:, c * 8:(c + 1) * 8] = (
            o[:, :8].view(ml_dtypes.bfloat16).astype(np.float32))
        # global window id g = 8*slot + core
        wins[:, c * 8:(c + 1) * 8] = 8 * o[:, 8:16].astype(np.int64) + c
    return vals, wins


def kernel(outlayer, c, train_ill, k):
    k = int(k)
    outlayer = np.asarray(outlayer, np.float32)
    train_ill = np.asarray(train_ill)
    X = np.ascontiguousarray(
        outlayer.transpose(1, 0, 2).reshape(N_, KD)).astype(np.float32)
    left = train_ill[:, 0].astype(np.int64)
    right = train_ill[:, 1].astype(np.int64)

    in_maps, order_norm = _prep_inputs(X, left, right)
    vals, wins = _mine(
        in_maps, trace=bool(int(os.environ.get("KNN_TRACE", "0"))))
    vals = vals[:2 * T_]
    wins = wins[:2 * T_]

    # host: pick SELW windows per query by device value, exact-recompute
    nq = 2 * T_
    sel = np.argpartition(-vals, SELW - 1, axis=1)[:, :SELW]     # [nq, SELW]
    gsel = np.take_along_axis(wins, sel, axis=1)                 # global wins
    cand_sorted = gsel[:, :, None] * W + np.arange(W)[None, None, :]
    cand_sorted = cand_sorted.reshape(nq, SELW * W)
    pad_mask = cand_sorted >= N_                                 # tail + pad
    cand = order_norm[np.clip(cand_sorted, 0, N_ - 1)]           # original ids
    q_idx = np.concatenate([right, left])

    X64 = X.astype(np.float64)
    sq = (X64 ** 2).sum(1)
    # exact sqdist via |q|^2 + |y|^2 - 2 q.y with per-chunk batched GEMV
    ncand = SELW * W
    B = np.empty((nq, ncand))
    step = 512
    for s in range(0, nq, step):
        e = min(s + step, nq)
        Y = X[cand[s:e]].astype(np.float64)                      # [b, nc, 512]
        G = np.einsum("bd,bcd->bc", X64[q_idx[s:e]], Y, optimize=True)
        B[s:e] = sq[q_idx[s:e], None] + sq[cand[s:e]] - 2.0 * G
    B[pad_mask] = np.inf
    B[cand == q_idx[:, None]] = np.inf                           # drop self
    B = np.sort(B, axis=1)[:, :k]                                # k NN dists

    D = ((X64[left] - X64[right]) ** 2).sum(1) + 1.0             # [t]
    B2 = B[:T_]                                                  # mining of right
    B1 = B[T_:]                                                  # mining of left
    L1 = np.maximum(D[:, None] - B1, 0.0)
    L2 = np.maximum(D[:, None] - B2, 0.0)
    loss = (L1.mean() + L2.mean()) / 2.0
    return np.asarray(loss, dtype=np.float32)
